# revision 13
# baseline (speedup 1.0000x reference)
"""CenterLoss Trainium2 kernel (raw Bass, 8-core SPMD).

loss = clip(distmat * onehot(label), 1e-12, 1e12).sum() / B
     = [ sum_b clip(||x_b - c_{label_b}||^2, 1e-12, 1e12) + B*(C-1)*1e-12 ] / B

Only the matching-class column of the masked distmat survives the one-hot
mask, so each core gathers the centers rows for its batch shard's labels
(indirect DMA from the full centers table in HBM), computes the per-sample
squared distance, and writes per-sample partials. The host sums the
per-core partials (the all-reduce of the scalar loss) and adds the
deterministic clamp constant contributed by the masked-off entries.
The clip itself is a numerical no-op on the surviving term (values are
O(2*FEAT_DIM) ~ 256, far inside [1e-12, 1e12]), so it is folded away.

Sharding: batch split across the 8 cores (128 samples each); the centers
table is visible to every core but only the 128 matching rows are read.

Structure (critical-path tuned against the CoreSim cost model):
- Both input DMAs (x shard on the ACT HWDGE queue, label offsets on the
  gpsimd/SWDGE queue) are issued in the function PREAMBLE, ahead of the
  Block entry bookkeeping, so their fixed DGE latency starts ticking
  immediately. The indirect centers gather chains behind the offsets
  load on the SWDGE ring (walrus requires its offsets to live in SBUF)
  and pays only its transfer slot, not a second DGE init.
- The compute chain is two DVE ops: tensor_tensor subtract, then a fused
  scalar_tensor_tensor multiply with row-accumulate (diff*diff with
  accum_out), yielding per-sample squared distances in one pass — half
  the instruction count of the subtract/mul/reduce/clamp chain.
- The 512B result store is an INDIRECT scatter (identity offsets from an
  iota) on the gpsimd/SWDGE ring, gated on x_sem reaching 17 (16 from the
  x-load DMA + 1 from the accumulate op). The scatter's completion sem is
  observed by the gpsimd engine's own-queue bookkeeping, so the engines
  all retire ~1.8us before the store's DGE pipeline drains and the exit
  barrier cost is fully hidden under the store latency; a sync-engine
  store would serialize the exit barrier after its completion instead.
  (A kv_writeback prepare/trigger store would remove the post-compute DGE
  init latency entirely, but its load_library reload does not compile on
  this walrus build.)
- Block(no_gpsimd_drain=True): every DMA completion sem is explicitly
  waited on (x_sem/gather_sem by DVE, st_sem by gpsimd), so the expensive
  pool dge_drain at block exit is redundant; the sem-only barrier plus the
  semaphore-context teardown (dma_reset + sem_clear per sem range) keeps
  warm-core re-execution hermetic.

Written in raw Bass (explicit semaphores) — the Tile kernel-tail drain
emits more sync waits per instruction than this walrus build accepts,
as do fused InstISA ops and InstPseudoReloadLibraryIndex.
"""

from contextlib import ExitStack

import numpy as np

import concourse.bass as bass
from concourse import mybir
from concourse.bass_utils import run_bass_kernel_spmd

B = 1024
D = 128
C = 100000
N_CORES = 8
P = 128
B_SHARD = B // N_CORES  # 128 samples per core


CLAMP_MIN = 1e-12

_prog_cache = {}


def build_nc() -> bass.Bass:
    nc = bass.Bass()
    x = nc.declare_dram_parameter("x", [B_SHARD, D], mybir.dt.float32, isOutput=False)
    cen = nc.declare_dram_parameter("cen", [C, D], mybir.dt.float32, isOutput=False)
    offs = nc.declare_dram_parameter(
        "offs", [B_SHARD, 1], mybir.dt.int32, isOutput=False
    )
    out = nc.declare_dram_parameter(
        "out", [B_SHARD, 1], mybir.dt.float32, isOutput=True
    )

    es = ExitStack()
    xt = es.enter_context(nc.sbuf_tensor([P, D], mybir.dt.float32))
    idx = es.enter_context(nc.sbuf_tensor([P, 1], mybir.dt.int32))
    ct = es.enter_context(nc.sbuf_tensor([P, D], mybir.dt.float32))
    diff = es.enter_context(nc.sbuf_tensor([P, D], mybir.dt.float32))
    junk = es.enter_context(nc.sbuf_tensor([P, D], mybir.dt.float32))
    sidx = es.enter_context(nc.sbuf_tensor([P, 1], mybir.dt.int32))
    res = es.enter_context(nc.sbuf_tensor([P, 1], mybir.dt.float32))
    x_sem = es.enter_context(nc.semaphore("x_sem"))
    idx_sem = es.enter_context(nc.semaphore("idx_sem"))
    gather_sem = es.enter_context(nc.semaphore("gather_sem"))
    vec_sem = es.enter_context(nc.semaphore("vec_sem"))
    misc_sem = es.enter_context(nc.semaphore("misc_sem"))
    st_sem = es.enter_context(nc.semaphore("st_sem"))

    # Preamble: input DMAs start ahead of the Block entry bookkeeping. The
    # x shard rides the ACT HWDGE queue; the label offsets ride the SWDGE
    # queue so the dependent indirect gather can chain behind them.
    nc.scalar.dma_start(out=xt[:], in_=x[:, :]).then_inc(x_sem, 16)
    nc.gpsimd.dma_start(out=idx[:], in_=offs[:, :]).then_inc(idx_sem, 16)
    nc.gpsimd.iota(
        sidx[:], pattern=[[0, 1]], base=0, channel_multiplier=1
    ).then_inc(misc_sem, 1)

    with nc.Block(no_gpsimd_drain=True) as block:

        @block.gpsimd
        def _(gpsimd):
            gpsimd.wait_ge(idx_sem, 16)
            gpsimd.indirect_dma_start(
                out=ct[:],
                out_offset=None,
                in_=cen[:],
                in_offset=bass.IndirectOffsetOnAxis(ap=idx[:, :1], axis=0),
            ).then_inc(gather_sem, 16)
            gpsimd.wait_ge(misc_sem, 1)
            gpsimd.wait_ge(x_sem, 17)
            gpsimd.indirect_dma_start(
                out=out[:, :],
                out_offset=bass.IndirectOffsetOnAxis(ap=sidx[:, :1], axis=0),
                in_=res[:],
                in_offset=None,
            ).then_inc(st_sem, 16)
            gpsimd.wait_ge(st_sem, 16)

        @block.vector
        def _(vector):
            vector.wait_ge(x_sem, 16)
            vector.wait_ge(gather_sem, 16)
            vector.tensor_tensor(
                out=diff[:], in0=xt[:], in1=ct[:], op=mybir.AluOpType.subtract
            ).then_inc(vec_sem, 1)
            vector.wait_ge(vec_sem, 1)
            vector.scalar_tensor_tensor(
                out=junk[:],
                in0=diff[:],
                scalar=1.0,
                in1=diff[:],
                op0=mybir.AluOpType.mult,
                op1=mybir.AluOpType.mult,
                accum_out=res[:],
            ).then_inc(x_sem, 1)

    es.close()
    return nc


def make_in_maps(input_x, input_label, centers):
    x = np.ascontiguousarray(np.asarray(input_x), dtype=np.float32)
    labels = np.asarray(input_label).astype(np.int64).ravel()
    cen = np.ascontiguousarray(np.asarray(centers), dtype=np.float32)
    assert x.shape == (B, D) and cen.shape == (C, D) and labels.shape == (B,)

    in_maps = []
    for k in range(N_CORES):
        lo = k * B_SHARD
        hi = lo + B_SHARD
        in_maps.append(
            {
                "x": x[lo:hi],
                "cen": cen,
                "offs": labels[lo:hi].astype(np.int32).reshape(B_SHARD, 1),
            }
        )
    return in_maps


def _finish(partials):
    total = np.float64(0.0)
    for p in partials:
        total += np.asarray(p, dtype=np.float64).sum()
    loss = (total + B * (C - 1) * CLAMP_MIN) / B
    return np.float32(loss)


def kernel(input_x, input_label, centers):
    if "nc" not in _prog_cache:
        _prog_cache["nc"] = build_nc()
    nc = _prog_cache["nc"]
    in_maps = make_in_maps(input_x, input_label, centers)
    res = run_bass_kernel_spmd(nc, in_maps, core_ids=list(range(N_CORES)))
    return _finish([r["out"] for r in res.results])
